# revision 1
# baseline (speedup 1.0000x reference)
"""Trainium2 Bass kernel for nn_BilinearUpsampling_88373247082947.

Math (from the reference):
    outer[b,t,:] = (w[0] * x[b,t,:]) ⊗ x[b,t,:]              # (C*C,) row
    normed       = outer * rsqrt(max(sum(outer^2), EPS))
    out          = repeat(normed, 2, axis=1)                  # (B, 2T, C*C)

Key simplification: sum(outer^2) over the C*C axis equals (w^2) * (sum(x^2))^2,
so the normalizer is a per-frame scalar computed from ||x||^2 — the outer
product never needs to be materialized before normalization.

Per-frame output row:  out_row[c*C + d] = s_t * x[t,c] * x[t,d]
with s_t = w * rsqrt(max(w^2 * n_t^2, EPS)),  n_t = sum_c x[t,c]^2.

Sharding: pure data parallel over batch — core b handles example b
(B=8 == n_cores). Each core writes its own (2T, C*C) = 64 MiB slice; the
kernel is HBM-write-bound (~512 MiB total output).

Device layout (per core): frames on partitions. For each tile of 128 frames:
  - n = rowsum(x^2)  (DVE tensor_tensor_reduce)
  - s = w / sqrt(max(w^2 n^2, EPS))  (small [128,1] ops)
  - xs = x * s       (per-partition scalar multiply)
  - for c in range(128): ot[:, c*128:(c+1)*128] = x * xs[:, c]   (DVE 2x mode)
  - DMA ot twice to DRAM (even/odd output rows), 64 KiB contiguous/partition.
"""

import sys

import numpy as np

if "/opt/trn_rl_repo" not in sys.path:
    sys.path.insert(0, "/opt/trn_rl_repo")

B = 8
T = 512
C = 128
STRIDE = 2
EPS = 1e-12
N_CORES = 8
TT = 128          # frames per SBUF tile
NT = T // TT      # tiles per core
CC = C * C

_CACHE = {}


def _build_nc():
    """Build and compile the per-core Bass program (SPMD: same NEFF on all cores)."""
    from contextlib import ExitStack

    import concourse.bacc as bacc
    import concourse.mybir as mybir
    import concourse.tile as tile

    f32 = mybir.dt.float32
    Alu = mybir.AluOpType

    nc = bacc.Bacc("TRN2", target_bir_lowering=False, debug=False)

    x_d = nc.dram_tensor("x", [T, C], f32, kind="ExternalInput")
    w_d = nc.dram_tensor("w", [C], f32, kind="ExternalInput")  # host-replicated w[0]
    o_d = nc.dram_tensor("out", [T * STRIDE, CC], f32, kind="ExternalOutput")

    x_ap = x_d.ap()
    w_ap = w_d.ap()
    o_ap = o_d.ap()

    # out row index = 2*(i*TT + p) + r  ->  [i, p, r, d] view
    o_v = o_ap.rearrange("(i p r) d -> i p r d", p=TT, r=STRIDE)
    # x row index = i*TT + p  ->  [p, i, c] view (partition-major per tile)
    x_v = x_ap.rearrange("(i p) c -> p i c", p=TT)

    NCH = 2                # output column-chunks per frame tile
    CH = CC // NCH         # elems per chunk (8192)
    CPC = C // NCH         # c-values per chunk (64)
    BC = 16                # c-values per wide tensor_tensor block
    NBLK = CPC // BC       # blocks per chunk (4)

    with tile.TileContext(nc) as tc, ExitStack() as ctx:
        const = ctx.enter_context(tc.tile_pool(name="const", bufs=1))
        small = ctx.enter_context(tc.tile_pool(name="small", bufs=1))
        outp = ctx.enter_context(tc.tile_pool(name="outp", bufs=3))

        x_all = const.tile([TT, NT, C], f32)
        nc.sync.dma_start(out=x_all[:, :, :], in_=x_v)

        w_bc = const.tile([TT, 1], f32)
        nc.sync.dma_start(out=w_bc[:, :], in_=w_ap.rearrange("(p c) -> p c", c=1))

        w2 = const.tile([TT, 1], f32)
        nc.vector.tensor_scalar(
            out=w2[:, :], in0=w_bc[:, :], scalar1=w_bc[:, 0:1], scalar2=None,
            op0=Alu.mult,
        )

        # Per-frame scale s = w / sqrt(max(w^2 n^2, EPS)), n = rowsum(x^2);
        # all tiles up front so the big loop below has no prep on the
        # critical path.
        xs_all = const.tile([TT, NT, C], f32)
        for i in range(NT):
            xt = x_all[:, i, :]
            sq = small.tile([TT, C], f32, tag="sq")
            n = small.tile([TT, 1], f32, tag="n")
            nc.vector.tensor_tensor(out=sq[:, :], in0=xt, in1=xt, op=Alu.mult)
            nc.vector.reduce_sum(
                out=n[:, :], in_=sq[:, :], axis=mybir.AxisListType.X
            )
            m = small.tile([TT, 1], f32, tag="m")
            nc.vector.tensor_scalar(
                out=m[:, :], in0=n[:, :], scalar1=n[:, 0:1], scalar2=None,
                op0=Alu.mult,
            )
            nc.vector.tensor_scalar(
                out=m[:, :], in0=m[:, :], scalar1=w2[:, 0:1], scalar2=EPS,
                op0=Alu.mult, op1=Alu.max,
            )
            rt = small.tile([TT, 1], f32, tag="rt")
            nc.scalar.sqrt(out=rt[:, :], in_=m[:, :])
            inv = small.tile([TT, 1], f32, tag="inv")
            nc.vector.reciprocal(out=inv[:, :], in_=rt[:, :])
            s = small.tile([TT, 1], f32, tag="s")
            nc.vector.tensor_scalar(
                out=s[:, :], in0=inv[:, :], scalar1=w_bc[:, 0:1], scalar2=None,
                op0=Alu.mult,
            )
            nc.vector.tensor_scalar(
                out=xs_all[:, i, :], in0=xt, scalar1=s[:, 0:1], scalar2=None,
                op0=Alu.mult,
            )

        # Outer products: och[p, c*C+d] = xs[p, c] * x[p, d] via
        # stride-0-broadcast tensor_tensor (BC c-values per instruction).
        # Steady state uses one full-row tile + two 8 MiB DMAs (64 KiB/
        # partition descriptors keep all 16 SDMA engines at full rate);
        # tile 0 alone is split into column halves so the first DMA can
        # start after half the compute.
        def emit_block(dst_tile, dst_off, i, c0):
            out_v = dst_tile[:, dst_off:dst_off + BC * C].rearrange(
                "p (c d) -> p c d", d=C
            )
            in0 = xs_all[:, i, c0:c0 + BC].unsqueeze(2).broadcast_to([TT, BC, C])
            in1 = x_all[:, i, :].unsqueeze(1).broadcast_to([TT, BC, C])
            nc.vector.tensor_tensor(out=out_v, in0=in0, in1=in1, op=Alu.mult)

        # Tile 0 drains in halves so the DMA chain starts after only half
        # the first tile's compute; later tiles use full 8 MiB DMAs whose
        # 64 KiB-per-partition descriptors run all 16 SDMA engines at rate.
        ot0 = outp.tile([TT, CC], f32, tag="full")
        for h in range(NCH):
            for k in range(NBLK):
                blk = h * NBLK + k
                emit_block(ot0, blk * BC * C, 0, blk * BC)
            for r in range(STRIDE):
                nc.sync.dma_start(
                    out=o_v[0, :, r, h * CH:(h + 1) * CH],
                    in_=ot0[:, h * CH:(h + 1) * CH],
                )

        for i in range(1, NT):
            ot = outp.tile([TT, CC], f32, tag="full")
            for k in range(C // BC):
                emit_block(ot, k * BC * C, i, k * BC)
            for r in range(STRIDE):
                nc.sync.dma_start(out=o_v[i, :, r, :], in_=ot[:, :])

    nc.compile()
    return nc


def _ensure_trace_support():
    """Install the NTFF profile hook that the image's antenv lacks.

    Only used by the dev/test harness (trace=True); the plain kernel() path
    never calls this.
    """
    import types

    import antenv

    if "antenv.axon_hooks" not in sys.modules:
        mod = types.ModuleType("antenv.axon_hooks")
        _state = {"hook": None}
        mod.set_axon_ntff_profile_hook = lambda h: _state.__setitem__("hook", h)
        mod.get_axon_ntff_profile_hook = lambda: _state["hook"]
        sys.modules["antenv.axon_hooks"] = mod
        antenv.axon_hooks = mod
    from antenv.axon_hooks import (
        get_axon_ntff_profile_hook,
        set_axon_ntff_profile_hook,
    )

    if get_axon_ntff_profile_hook() is None:
        from trn_agent_boot.trn_boot import _ntff_profile_via_ctypes

        set_axon_ntff_profile_hook(
            _ntff_profile_via_ctypes("/opt/axon/libaxon_pjrt.so")
        )
    import concourse.bass_utils as bu

    bu.upload_artifacts = lambda tmpdir: tmpdir


def _run(inputs, trace=False, **spmd_kwargs):
    """Shard, run on 8 cores, gather. Returns (full_output, BassKernelResults)."""
    from concourse.bass_utils import run_bass_kernel_spmd

    if trace:
        _ensure_trace_support()

    if "nc" not in _CACHE:
        _CACHE["nc"] = _build_nc()
    nc = _CACHE["nc"]

    x = np.ascontiguousarray(np.asarray(inputs["x"], dtype=np.float32))
    w = np.asarray(inputs["w"], dtype=np.float32).reshape(-1)
    assert x.shape == (B, T, C), x.shape
    w_rep = np.full((C,), w[0], dtype=np.float32)

    in_maps = [{"x": x[b], "w": w_rep} for b in range(N_CORES)]
    res = run_bass_kernel_spmd(
        nc, in_maps, core_ids=list(range(N_CORES)), trace=trace, **spmd_kwargs
    )
    out = np.stack([res.results[b]["out"] for b in range(N_CORES)], axis=0)
    return out, res


def kernel(**inputs) -> np.ndarray:
    out, _ = _run(inputs)
    return out



# revision 5
# speedup vs baseline: 1.0493x; 1.0493x over previous
"""Trainium2 Bass kernel for nn_BilinearUpsampling_88373247082947.

Math (from the reference):
    outer[b,t,:] = (w[0] * x[b,t,:]) ⊗ x[b,t,:]              # (C*C,) row
    normed       = outer * rsqrt(max(sum(outer^2), EPS))
    out          = repeat(normed, 2, axis=1)                  # (B, 2T, C*C)

Key simplification: sum(outer^2) over the C*C axis equals (w^2) * (sum(x^2))^2,
so the normalizer is a per-frame scalar computed from ||x||^2 — the outer
product never needs to be materialized before normalization.

Per-frame output row:  out_row[c*C + d] = s_t * x[t,c] * x[t,d]
with s_t = w * rsqrt(max(w^2 * n_t^2, EPS)),  n_t = sum_c x[t,c]^2.


Sharding: pure data parallel over batch — core b handles example b
(B=8 == n_cores). Each core writes its own (2T, C*C) = 64 MiB slice.

Perf model (from NTFF traces): all 16 SDMA engines stream at their ~27 GB/s
ceiling (432 GB/s/core aggregate, zero HBM backpressure) once output DMAs
start, so total time = ramp + 64 MiB / 432 GB/s + ~2.8 us tail. The kernel
is structured to minimize the ramp (time to first output descriptor):
  - one packed input tensor [w | x tile0 | x tiles1-3], loaded by two DMAs
    (tile0+w first) with large contiguous per-partition descriptors;
  - tile-0 scale prep on a short 5-op critical chain (ACT square+accum ->
    DVE fused w^2*n^2 -> ACT sqrt(+eps) -> DVE reciprocal -> DVE fused xs);
  - tile 0's outer product emitted in progressive column chunks
    [4,4,8,16,32,64] c-values, each chunk DMA'd immediately (r=0 copy on
    the sync HWDGE queue, r=1 on the scalar HWDGE queue, so issue overhead
    runs on two engines in parallel);
  - tiles 1-3 computed in halves, DMA'd per half the same dual-queue way.
"""

import sys

import numpy as np

if "/opt/trn_rl_repo" not in sys.path:
    sys.path.insert(0, "/opt/trn_rl_repo")

B = 8
T = 512
C = 128
STRIDE = 2
EPS = 1e-12
N_CORES = 8
TT = 128          # frames per SBUF tile
NT = T // TT      # tiles per core
CC = C * C

_CACHE = {}


def _build_nc():
    """Build and compile the per-core Bass program (SPMD: same NEFF on all cores)."""
    from contextlib import ExitStack

    import concourse.bacc as bacc
    import concourse.mybir as mybir
    import concourse.tile as tile

    f32 = mybir.dt.float32
    Alu = mybir.AluOpType
    Act = mybir.ActivationFunctionType

    nc = bacc.Bacc("TRN2", target_bir_lowering=False, debug=False)

    # Packed input: col 0 = w (host-replicated), cols 1+i*C+c = x[i*TT+p, c].
    xin_d = nc.dram_tensor("xin", [TT, 1 + NT * C], f32, kind="ExternalInput")
    o_d = nc.dram_tensor("out", [T * STRIDE, CC], f32, kind="ExternalOutput")

    xin_ap = xin_d.ap()
    o_ap = o_d.ap()

    # out row index = 2*(i*TT + p) + r  ->  [i, p, r, d] view
    o_v = o_ap.rearrange("(i p r) d -> i p r d", p=TT, r=STRIDE)

    CH = CC // 2           # elems per half-tile output chunk (8192)
    BC = 16                # c-values per wide tensor_tensor block (tiles 1-3)
    CHUNKS0 = [4, 4, 8, 16, 32, 64]   # progressive c-chunks for tile 0

    with tile.TileContext(nc) as tc, ExitStack() as ctx:
        const = ctx.enter_context(tc.tile_pool(name="const", bufs=1))
        small = ctx.enter_context(tc.tile_pool(name="small", bufs=1))
        outp = ctx.enter_context(tc.tile_pool(name="outp", bufs=3))

        xbuf = const.tile([TT, 1 + NT * C], f32)
        # Tile-0 slice (+w) first so its prep starts ASAP; rest in parallel
        # on the scalar HWDGE queue.
        nc.sync.dma_start(out=xbuf[:, 0:1 + C], in_=xin_ap[:, 0:1 + C])
        nc.scalar.dma_start(out=xbuf[:, 1 + C:], in_=xin_ap[:, 1 + C:])

        w = xbuf[:, 0:1]
        x_t = [xbuf[:, 1 + i * C:1 + (i + 1) * C] for i in range(NT)]

        w2 = small.tile([TT, 1], f32, tag="w2")
        nc.vector.tensor_scalar(
            out=w2[:, :], in0=w, scalar1=w[:, 0:1], scalar2=None, op0=Alu.mult,
        )

        xs_all = const.tile([TT, NT * C], f32)

        def prep(i):
            """xs[i] = x[i] * w * rsqrt(w^2 * rowsum(x[i]^2)^2 + EPS)."""
            sq = small.tile([TT, C], f32, tag=f"sq{i % 2}")
            n = small.tile([TT, 1], f32, tag=f"n{i}")
            nc.scalar.activation(
                out=sq[:, :], in_=x_t[i], func=Act.Square, accum_out=n[:, :],
            )
            v = small.tile([TT, 1], f32, tag=f"v{i}")
            nc.vector.tensor_scalar(
                out=v[:, :], in0=n[:, :], scalar1=n[:, 0:1], scalar2=None,
                op0=Alu.mult,
            )
            m = small.tile([TT, 1], f32, tag=f"m{i}")
            nc.vector.tensor_scalar(
                out=m[:, :], in0=v[:, :], scalar1=w2[:, 0:1], scalar2=EPS,
                op0=Alu.mult, op1=Alu.max,
            )
            rt = small.tile([TT, 1], f32, tag=f"rt{i}")
            nc.scalar.activation(out=rt[:, :], in_=m[:, :], func=Act.Sqrt)
            inv = small.tile([TT, 1], f32, tag=f"inv{i}")
            nc.vector.reciprocal(out=inv[:, :], in_=rt[:, :])
            nc.vector.tensor_scalar(
                out=xs_all[:, i * C:(i + 1) * C], in0=x_t[i],
                scalar1=inv[:, 0:1], scalar2=w[:, 0:1],
                op0=Alu.mult, op1=Alu.mult,
            )

        def emit_block(dst_tile, i, c0, bc):
            """dst[:, c*C+d] = xs[c0+c]*x[d] for c in [0,bc) via broadcast mult."""
            out_v = dst_tile[:, c0 * C:(c0 + bc) * C].rearrange(
                "p (c d) -> p c d", d=C
            )
            in0 = (
                xs_all[:, i * C + c0:i * C + c0 + bc]
                .unsqueeze(2).broadcast_to([TT, bc, C])
            )
            in1 = x_t[i].unsqueeze(1).broadcast_to([TT, bc, C])
            nc.vector.tensor_tensor(out=out_v, in0=in0, in1=in1, op=Alu.mult)

        def emit_dmas(ot, i, e0, e1):
            """DMA ot cols [e0,e1) to both repeat rows, one HWDGE queue each."""
            nc.sync.dma_start(out=o_v[i, :, 0, e0:e1], in_=ot[:, e0:e1])
            nc.scalar.dma_start(out=o_v[i, :, 1, e0:e1], in_=ot[:, e0:e1])

        # Tile 0: progressive chunks so the first output DMA fires after only
        # a few c-values of compute.
        prep(0)
        ot0 = outp.tile([TT, CC], f32, tag="full")
        c0 = 0
        for bc in CHUNKS0:
            emit_block(ot0, 0, c0, bc)
            emit_dmas(ot0, 0, c0 * C, (c0 + bc) * C)
            c0 += bc

        # Tiles 1-3: halves (the DMA queues are backlogged by now; halves keep
        # buffer recycling and issue overhead comfortably ahead of the drain).
        for i in range(1, NT):
            prep(i)
            ot = outp.tile([TT, CC], f32, tag="full")
            for h in range(2):
                for k in range(C // (2 * BC)):
                    emit_block(ot, i, h * (C // 2) + k * BC, BC)
                emit_dmas(ot, i, h * CH, (h + 1) * CH)

    nc.compile()
    return nc


def _ensure_trace_support():
    """Install the NTFF profile hook that the image's antenv lacks.

    Only used by the dev/test harness (trace=True); the plain kernel() path
    never calls this.
    """
    import types

    import antenv

    if "antenv.axon_hooks" not in sys.modules:
        mod = types.ModuleType("antenv.axon_hooks")
        _state = {"hook": None}
        mod.set_axon_ntff_profile_hook = lambda h: _state.__setitem__("hook", h)
        mod.get_axon_ntff_profile_hook = lambda: _state["hook"]
        sys.modules["antenv.axon_hooks"] = mod
        antenv.axon_hooks = mod
    from antenv.axon_hooks import (
        get_axon_ntff_profile_hook,
        set_axon_ntff_profile_hook,
    )

    if get_axon_ntff_profile_hook() is None:
        from trn_agent_boot.trn_boot import _ntff_profile_via_ctypes

        set_axon_ntff_profile_hook(
            _ntff_profile_via_ctypes("/opt/axon/libaxon_pjrt.so")
        )
    import concourse.bass_utils as bu

    bu.upload_artifacts = lambda tmpdir: tmpdir


def _run(inputs, trace=False, **spmd_kwargs):
    """Shard, run on 8 cores, gather. Returns (full_output, BassKernelResults)."""
    from concourse.bass_utils import run_bass_kernel_spmd

    if trace:
        _ensure_trace_support()

    if "nc" not in _CACHE:
        _CACHE["nc"] = _build_nc()
    nc = _CACHE["nc"]

    x = np.ascontiguousarray(np.asarray(inputs["x"], dtype=np.float32))
    w = np.asarray(inputs["w"], dtype=np.float32).reshape(-1)
    assert x.shape == (B, T, C), x.shape

    # Pack per-core input: [w | x rows by partition] so one contiguous
    # per-partition DMA covers tile0+w and another covers tiles 1-3.
    xp = np.empty((B, TT, 1 + NT * C), dtype=np.float32)
    xp[:, :, 0] = w[0]
    xp[:, :, 1:] = x.reshape(B, NT, TT, C).transpose(0, 2, 1, 3).reshape(
        B, TT, NT * C
    )

    in_maps = [{"xin": xp[b]} for b in range(N_CORES)]
    res = run_bass_kernel_spmd(
        nc, in_maps, core_ids=list(range(N_CORES)), trace=trace, **spmd_kwargs
    )
    out = np.stack([res.results[b]["out"] for b in range(N_CORES)], axis=0)
    return out, res


def kernel(**inputs) -> np.ndarray:
    out, _ = _run(inputs)
    return out
